# revision 27
# baseline (speedup 1.0000x reference)
"""BlockHadamardDPD kernel for 8x Trainium2 NeuronCores (Bass/Tile).

y = ((x reshaped [., 64] @ H64/8) reshaped back) * sign1, permuted by perm, * sign2

The op is linear along dim:  y[t, j] = sum_k x[t, k] * M[k, j] with
M = blockdiag(H64/8) * diag(s1), columns gathered by perm, * diag(s2).
Since perm/signs are host-visible inputs, fold both sign vectors into the
block-diagonal weight (entries stay exactly +-1/8) and apply the column
permutation during the host-side unshard gather.

Device work per core (1 batch of [4096 tok, 4096 dim], data-parallel):
  z^T = blockdiag(W_c) @ x^T   --  32 chunks of 128 dims, stationary-weight
  matmuls [k=128, m=128, n=512 tok], fp32 PSUM accumulate, fp16 out.
Input x is quantized host-side to fp8 e3m4 with a per-(token, 64-block)
absmax scale (the Hadamard only mixes within a 64-block, so the scale
factors out and is re-applied during the host unshard). HBM traffic is
16MB in + 32MB out per core.

Layout: dims split into chunks of 128 partitions; host pre-packs xt so
each input superstep (8 chunks) and output superstep (4 chunks) is ONE
contiguous 4MB DMA.
"""
import sys
sys.path.insert(0, "/opt/trn_rl_repo")
import numpy as np
import ml_dtypes

B, S, D = 8, 4096, 4096
BLOCK = 64
NCORES = 8
C, R = 32, 128          # chunks x rows (dim = C*R)
SIN = 8                 # chunks per input piece (one 4MB fp8 DMA)
SOUT = 4                # chunks per output superstep (one 4MB fp16 DMA)
TOK = 512               # moving free dim per matmul (one PSUM bank fp32)
QMAX = 15.5             # e3m4 max normal

_nc_cache = []
_w_cache = {}
_last_run = None


def _hadamard(n):
    H = np.array([[1.0]], dtype=np.float64)
    base = np.array([[1.0, 1.0], [1.0, -1.0]], dtype=np.float64)
    while H.shape[0] < n:
        H = np.kron(H, base)
    return H


def _build_weights(perm, sign1, sign2):
    """w_p[k, c*128+m] = H2[k, m] * s1[c*128+m] * s2[o(c*128+m)], e3m4."""
    perm = np.asarray(perm).astype(np.int64)
    o = np.empty(D, np.int64)
    o[perm] = np.arange(D)
    w_vec = np.asarray(sign1, np.float64) * np.asarray(sign2, np.float64)[o]
    H64 = _hadamard(BLOCK) / np.sqrt(float(BLOCK))
    H2 = np.zeros((R, R))
    H2[:64, :64] = H64
    H2[64:, 64:] = H64
    W = H2[None, :, :] * w_vec.reshape(C, 1, R)   # [c, k, m]
    w_p = W.transpose(1, 0, 2).reshape(R, C * R)  # [k, c*R+m]
    return np.ascontiguousarray(w_p).astype(ml_dtypes.float8_e3m4)


def _build_nc():
    import concourse.bacc as bacc
    import concourse.mybir as mybir
    import concourse.tile_utils as tile_utils
    tile_utils.max_sbuf_usage = 206 * 1024
    from concourse.tile import TileContext

    f8 = mybir.dt.float8e3
    f16 = mybir.dt.float16
    f32 = mybir.dt.float32
    nc = bacc.Bacc("TRN2", target_bir_lowering=False, debug=False,
                   num_devices=NCORES)
    xt = nc.dram_tensor("xt", [C // SIN, R, SIN * S], f8, kind="ExternalInput")
    w = nc.dram_tensor("w", [R, C * R], f8, kind="ExternalInput")
    yt = nc.dram_tensor("yt", [R, C * S], f16, kind="ExternalOutput")

    # output store groups: taper the tail so the last stores overlap the
    # final chunks' evacuation, and so the tail groups' buffer-rotation
    # waits land on small stores rather than 4MB ones
    groups = [(g * SOUT, SOUT) for g in range(6)] + \
             [(24, 2), (26, 2), (28, 2), (30, 1), (31, 1)]

    with TileContext(nc) as tc:
        with tc.tile_pool(name="wp", bufs=1) as wp, \
             tc.tile_pool(name="xin", bufs=3) as xin, \
             tc.tile_pool(name="yout", bufs=3) as yo, \
             tc.tile_pool(name="ps", bufs=8, space="PSUM") as ps:
            w_sb = wp.tile([R, C * R], f8, tag="wsb", name="wsb")
            # weights on the scalar HWDGE ring (overlaps the first x load
            # on the sync ring); output stores also on the scalar ring so
            # input loads never queue behind store completions. x comes in
            # 2MB pieces, all queued upfront on the sync ring: FIFO order
            # keeps them sequential, buffer rotation (bufs=3) self-paces
            # the ring, and the small first piece lets compute - and the
            # store stream - start ~20us earlier than one big load
            nc.scalar.dma_start(out=w_sb[:, :], in_=w.ap()[:, :])
            xs_tiles = []
            for si in range(C // SIN):
                xst = xin.tile([R, SIN * S], f8, tag="xs", name=f"xs{si}")
                # piece 0 rides the scalar ring right behind the weights:
                # it starts streaming while the sync ring is still in its
                # preamble, hiding the ~3us startup gap
                eng = nc.scalar if si == 0 else nc.sync
                eng.dma_start(out=xst[:, :], in_=xt.ap()[si, :, :])
                xs_tiles.append(xst)
            ncopy = 0
            for gi, (c0, ng) in enumerate(groups):
                ys = yo.tile([R, ng * S], f16, tag="ys", name=f"ys{gi}")
                for j in range(ng):
                    c = c0 + j
                    xs = xs_tiles[c // SIN]
                    jj = c % SIN
                    for b in range(S // TOK):
                        pt = ps.tile([R, TOK], f32, tag="pt", name=f"pt{c}_{b}")
                        nc.tensor.matmul(pt[:, :],
                                         w_sb[:, c * R:(c + 1) * R],
                                         xs[:, jj * S + b * TOK:
                                            jj * S + (b + 1) * TOK])
                        dst = ys[:, j * S + b * TOK:j * S + (b + 1) * TOK]
                        if ncopy % 2 == 0:
                            nc.vector.tensor_copy(dst, pt[:, :])
                        else:
                            nc.scalar.copy(out=dst, in_=pt[:, :])
                        ncopy += 1
                nc.scalar.dma_start(
                    out=yt.ap()[:, c0 * S:(c0 + ng) * S], in_=ys[:, :])
    nc.compile()
    return nc


def kernel(x, sign1, sign2, perm):
    global _last_run
    x = np.asarray(x)
    sign1 = np.asarray(sign1)
    sign2 = np.asarray(sign2)
    perm = np.asarray(perm)

    if not _nc_cache:
        _nc_cache.append(_build_nc())
    nc = _nc_cache[0]

    key = (perm.tobytes(), sign1.tobytes(), sign2.tobytes())
    if key not in _w_cache:
        _w_cache[key] = _build_weights(perm, sign1, sign2)
    w_p = _w_cache[key]

    # host staging: per-(token, 64-block) absmax scale, quantize to e3m4,
    # transpose to [dim, tok] and pack for contiguous superstep DMAs
    in_maps = []
    scales = []
    for b in range(B):
        xr = x[b].astype(np.float32).reshape(S, D // BLOCK, BLOCK)
        amax = np.abs(xr).max(axis=2, keepdims=True)
        sc = np.maximum(amax / QMAX, 1e-8).astype(np.float32)
        xq = (xr / sc).astype(ml_dtypes.float8_e3m4).reshape(S, D)
        scales.append(sc.reshape(S, D // BLOCK))
        xt_dev = np.ascontiguousarray(
            xq.reshape(S, C // SIN, SIN, R).transpose(1, 3, 2, 0)
        ).reshape(C // SIN, R, SIN * S)
        in_maps.append({"xt": xt_dev, "w": w_p})

    from concourse.bass_utils import run_bass_kernel_spmd
    res = run_bass_kernel_spmd(nc, in_maps, list(range(NCORES)))
    _last_run = (nc, in_maps)

    perm64 = perm.astype(np.int64)
    blk = perm64 >> 6                      # source 64-block of output col j
    out = np.empty((B, S, D), dtype=np.float32)
    for b in range(B):
        yt_dev = np.asarray(res.results[b]["yt"]).reshape(R, C, S)
        zT = yt_dev.transpose(1, 0, 2).reshape(D, S)
        g = zT[perm64].astype(np.float32)          # [Dout, S]
        scT = np.ascontiguousarray(scales[b].T)    # [D//BLOCK, S]
        g *= scT[blk]
        out[b] = g.T
    return out


# revision 29
# speedup vs baseline: 1.1357x; 1.1357x over previous
"""BlockHadamardDPD kernel for 8x Trainium2 NeuronCores (Bass/Tile).

y = ((x reshaped [., 64] @ H64/8) reshaped back) * sign1, permuted by perm, * sign2

The op is linear along dim:  y[t, j] = sum_k x[t, k] * M[k, j] with
M = blockdiag(H64/8) * diag(s1), columns gathered by perm, * diag(s2).
Since perm/signs are host-visible inputs, fold both sign vectors into the
block-diagonal weight (entries stay exactly +-1/8) and apply the column
permutation during the host-side unshard gather.

Device work per core (1 batch of [4096 tok, 4096 dim], data-parallel):
  z^T = blockdiag(W_c) @ x^T   --  32 chunks of 128 dims, stationary-weight
  matmuls [k=128, m=128, n=512 tok], fp32 PSUM accumulate, fp16 out.
Input x is quantized host-side to fp8 e3m4 with a per-(token, 64-block)
absmax scale (the Hadamard only mixes within a 64-block, so the scale
factors out and is re-applied during the host unshard). HBM traffic is
16MB in + 32MB out per core.

Layout: dims split into chunks of 128 partitions; host pre-packs xt so
each input superstep (8 chunks) and output superstep (4 chunks) is ONE
contiguous 4MB DMA.
"""
import sys
sys.path.insert(0, "/opt/trn_rl_repo")
import numpy as np
import ml_dtypes

B, S, D = 8, 4096, 4096
BLOCK = 64
NCORES = 8
C, R = 32, 128          # chunks x rows (dim = C*R)
SIN = 8                 # chunks per input piece (one 4MB fp8 DMA)
SOUT = 4                # chunks per output superstep (one 4MB fp16 DMA)
TOK = 512               # moving free dim per matmul (one PSUM bank fp32)
QMAX = 15.5             # e3m4 max normal

_nc_cache = []
_w_cache = {}
_last_run = None


def _hadamard(n):
    H = np.array([[1.0]], dtype=np.float64)
    base = np.array([[1.0, 1.0], [1.0, -1.0]], dtype=np.float64)
    while H.shape[0] < n:
        H = np.kron(H, base)
    return H


def _build_weights(perm, sign1, sign2):
    """w_p[k, c*128+m] = H2[k, m] * s1[c*128+m] * s2[o(c*128+m)], e3m4."""
    perm = np.asarray(perm).astype(np.int64)
    o = np.empty(D, np.int64)
    o[perm] = np.arange(D)
    w_vec = np.asarray(sign1, np.float64) * np.asarray(sign2, np.float64)[o]
    H64 = _hadamard(BLOCK) / np.sqrt(float(BLOCK))
    H2 = np.zeros((R, R))
    H2[:64, :64] = H64
    H2[64:, 64:] = H64
    W = H2[None, :, :] * w_vec.reshape(C, 1, R)   # [c, k, m]
    w_p = W.transpose(1, 0, 2).reshape(R, C * R)  # [k, c*R+m]
    return np.ascontiguousarray(w_p).astype(ml_dtypes.float8_e3m4)


def _build_nc():
    import concourse.bacc as bacc
    import concourse.mybir as mybir
    import concourse.tile_utils as tile_utils
    tile_utils.max_sbuf_usage = 206 * 1024
    from concourse.tile import TileContext

    f8 = mybir.dt.float8e3
    f16 = mybir.dt.float16
    f32 = mybir.dt.float32
    nc = bacc.Bacc("TRN2", target_bir_lowering=False, debug=False,
                   num_devices=NCORES)
    xt = nc.dram_tensor("xt", [C // SIN, R, SIN * S], f8, kind="ExternalInput")
    w = nc.dram_tensor("w", [R, C * R], f8, kind="ExternalInput")
    yt = nc.dram_tensor("yt", [R, C * S], f16, kind="ExternalOutput")

    # output store groups: taper the tail so the last stores overlap the
    # final chunks' evacuation, and so the tail groups' buffer-rotation
    # waits land on small stores rather than 4MB ones
    groups = [(g * SOUT, SOUT) for g in range(6)] + \
             [(24, 2), (26, 2), (28, 2), (30, 1), (31, 1)]

    with TileContext(nc) as tc:
        with tc.tile_pool(name="wp", bufs=1) as wp, \
             tc.tile_pool(name="xin", bufs=3) as xin, \
             tc.tile_pool(name="yout", bufs=3) as yo, \
             tc.tile_pool(name="ps", bufs=8, space="PSUM") as ps:
            w_sb = wp.tile([R, C * R], f8, tag="wsb", name="wsb")
            # weights on the scalar HWDGE ring (overlaps the first x load
            # on the sync ring); output stores also on the scalar ring so
            # input loads never queue behind store completions. x comes in
            # 2MB pieces, all queued upfront on the sync ring: FIFO order
            # keeps them sequential, buffer rotation (bufs=3) self-paces
            # the ring, and the small first piece lets compute - and the
            # store stream - start ~20us earlier than one big load
            nc.scalar.dma_start(out=w_sb[:, :], in_=w.ap()[:, :])
            xs_tiles = []
            for si in range(C // SIN):
                xst = xin.tile([R, SIN * S], f8, tag="xs", name=f"xs{si}")
                nc.sync.dma_start(out=xst[:, :], in_=xt.ap()[si, :, :])
                xs_tiles.append(xst)
            ncopy = 0
            for gi, (c0, ng) in enumerate(groups):
                ys = yo.tile([R, ng * S], f16, tag="ys", name=f"ys{gi}")
                for j in range(ng):
                    c = c0 + j
                    xs = xs_tiles[c // SIN]
                    jj = c % SIN
                    for b in range(S // TOK):
                        pt = ps.tile([R, TOK], f32, tag="pt", name=f"pt{c}_{b}")
                        nc.tensor.matmul(pt[:, :],
                                         w_sb[:, c * R:(c + 1) * R],
                                         xs[:, jj * S + b * TOK:
                                            jj * S + (b + 1) * TOK])
                        dst = ys[:, j * S + b * TOK:j * S + (b + 1) * TOK]
                        if ncopy % 2 == 0:
                            nc.vector.tensor_copy(dst, pt[:, :])
                        else:
                            nc.scalar.copy(out=dst, in_=pt[:, :])
                        ncopy += 1
                nc.scalar.dma_start(
                    out=yt.ap()[:, c0 * S:(c0 + ng) * S], in_=ys[:, :])
    nc.compile()
    return nc


def kernel(x, sign1, sign2, perm):
    global _last_run
    x = np.asarray(x)
    sign1 = np.asarray(sign1)
    sign2 = np.asarray(sign2)
    perm = np.asarray(perm)

    if not _nc_cache:
        _nc_cache.append(_build_nc())
    nc = _nc_cache[0]

    key = (perm.tobytes(), sign1.tobytes(), sign2.tobytes())
    if key not in _w_cache:
        _w_cache[key] = _build_weights(perm, sign1, sign2)
    w_p = _w_cache[key]

    # host staging: per-(token, 64-block) absmax scale, quantize to e3m4,
    # transpose to [dim, tok] and pack for contiguous superstep DMAs
    in_maps = []
    scales = []
    for b in range(B):
        xr = x[b].astype(np.float32).reshape(S, D // BLOCK, BLOCK)
        amax = np.abs(xr).max(axis=2, keepdims=True)
        sc = np.maximum(amax / QMAX, 1e-8).astype(np.float32)
        xq = (xr / sc).astype(ml_dtypes.float8_e3m4).reshape(S, D)
        scales.append(sc.reshape(S, D // BLOCK))
        xt_dev = np.ascontiguousarray(
            xq.reshape(S, C // SIN, SIN, R).transpose(1, 3, 2, 0)
        ).reshape(C // SIN, R, SIN * S)
        in_maps.append({"xt": xt_dev, "w": w_p})

    from concourse.bass_utils import run_bass_kernel_spmd
    res = run_bass_kernel_spmd(nc, in_maps, list(range(NCORES)))
    _last_run = (nc, in_maps)

    perm64 = perm.astype(np.int64)
    blk = perm64 >> 6                      # source 64-block of output col j
    out = np.empty((B, S, D), dtype=np.float32)
    for b in range(B):
        yt_dev = np.asarray(res.results[b]["yt"]).reshape(R, C, S)
        zT = yt_dev.transpose(1, 0, 2).reshape(D, S)
        g = zT[perm64].astype(np.float32)          # [Dout, S]
        scT = np.ascontiguousarray(scales[b].T)    # [D//BLOCK, S]
        g *= scT[blk]
        out[b] = g.T
    return out


# revision 30
# speedup vs baseline: 1.3004x; 1.1451x over previous
"""BlockHadamardDPD kernel for 8x Trainium2 NeuronCores (Bass/Tile).

y = ((x reshaped [., 64] @ H64/8) reshaped back) * sign1, permuted by perm, * sign2

The op is linear along dim:  y[t, j] = sum_k x[t, k] * M[k, j] with
M = blockdiag(H64/8) * diag(s1), columns gathered by perm, * diag(s2).
Since perm/signs are host-visible inputs, fold both sign vectors into the
block-diagonal weight (entries stay exactly +-1/8) and apply the column
permutation during the host-side unshard gather.

Device work per core (1 batch of [4096 tok, 4096 dim], data-parallel):
  z^T = blockdiag(W_c) @ x^T   --  32 chunks of 128 dims, stationary-weight
  matmuls [k=128, m=128, n=512 tok], fp32 PSUM accumulate, fp16 out.
Input x is quantized host-side to fp8 e3m4 with a per-(token, 64-block)
absmax scale (the Hadamard only mixes within a 64-block, so the scale
factors out and is re-applied during the host unshard). HBM traffic is
16MB in + 32MB out per core.

Layout: dims split into chunks of 128 partitions; host pre-packs xt so
each input superstep (8 chunks) and output superstep (4 chunks) is ONE
contiguous 4MB DMA.
"""
import sys
sys.path.insert(0, "/opt/trn_rl_repo")
import numpy as np
import ml_dtypes

B, S, D = 8, 4096, 4096
BLOCK = 64
NCORES = 8
C, R = 32, 128          # chunks x rows (dim = C*R)
SIN = 8                 # chunks per input piece (one 4MB fp8 DMA)
SOUT = 4                # chunks per output superstep (one 4MB fp16 DMA)
TOK = 512               # moving free dim per matmul (one PSUM bank fp32)
QMAX = 15.5             # e3m4 max normal

_nc_cache = []
_w_cache = {}
_last_run = None


def _hadamard(n):
    H = np.array([[1.0]], dtype=np.float64)
    base = np.array([[1.0, 1.0], [1.0, -1.0]], dtype=np.float64)
    while H.shape[0] < n:
        H = np.kron(H, base)
    return H


def _build_weights(perm, sign1, sign2):
    """w_p[k, c*128+m] = H2[k, m] * s1[c*128+m] * s2[o(c*128+m)], e3m4."""
    perm = np.asarray(perm).astype(np.int64)
    o = np.empty(D, np.int64)
    o[perm] = np.arange(D)
    w_vec = np.asarray(sign1, np.float64) * np.asarray(sign2, np.float64)[o]
    H64 = _hadamard(BLOCK) / np.sqrt(float(BLOCK))
    H2 = np.zeros((R, R))
    H2[:64, :64] = H64
    H2[64:, 64:] = H64
    W = H2[None, :, :] * w_vec.reshape(C, 1, R)   # [c, k, m]
    w_p = W.transpose(1, 0, 2).reshape(R, C * R)  # [k, c*R+m]
    return np.ascontiguousarray(w_p).astype(ml_dtypes.float8_e3m4)


def _build_nc():
    import concourse.bacc as bacc
    import concourse.mybir as mybir
    import concourse.tile_utils as tile_utils
    tile_utils.max_sbuf_usage = 206 * 1024
    from concourse.tile import TileContext

    f8 = mybir.dt.float8e3
    f16 = mybir.dt.float16
    f32 = mybir.dt.float32
    nc = bacc.Bacc("TRN2", target_bir_lowering=False, debug=False,
                   num_devices=NCORES)
    xt = nc.dram_tensor("xt", [C // SIN, R, SIN * S], f8, kind="ExternalInput")
    w = nc.dram_tensor("w", [R, C * R], f8, kind="ExternalInput")
    yt = nc.dram_tensor("yt", [R, C * S], f16, kind="ExternalOutput")

    # output store groups: taper the tail so the last stores overlap the
    # final chunks' evacuation instead of draining after it
    groups = [(g * SOUT, SOUT) for g in range(7)] + [(28, 2), (30, 1), (31, 1)]

    with TileContext(nc) as tc:
        with tc.tile_pool(name="wp", bufs=1) as wp, \
             tc.tile_pool(name="xin", bufs=3) as xin, \
             tc.tile_pool(name="yout", bufs=3) as yo, \
             tc.tile_pool(name="ps", bufs=8, space="PSUM") as ps:
            w_sb = wp.tile([R, C * R], f8, tag="wsb", name="wsb")
            # weights on the scalar HWDGE ring (overlaps the first x load
            # on the sync ring); output stores also on the scalar ring so
            # input loads never queue behind store completions. x comes in
            # 2MB pieces, all queued upfront on the sync ring: FIFO order
            # keeps them sequential, buffer rotation (bufs=3) self-paces
            # the ring, and the small first piece lets compute - and the
            # store stream - start ~20us earlier than one big load
            nc.scalar.dma_start(out=w_sb[:, :], in_=w.ap()[:, :])
            xs_tiles = []
            for si in range(C // SIN):
                xst = xin.tile([R, SIN * S], f8, tag="xs", name=f"xs{si}")
                nc.sync.dma_start(out=xst[:, :], in_=xt.ap()[si, :, :])
                xs_tiles.append(xst)
            ncopy = 0
            for gi, (c0, ng) in enumerate(groups):
                ys = yo.tile([R, ng * S], f16, tag="ys", name=f"ys{gi}")
                for j in range(ng):
                    c = c0 + j
                    xs = xs_tiles[c // SIN]
                    jj = c % SIN
                    for b in range(S // TOK):
                        pt = ps.tile([R, TOK], f32, tag="pt", name=f"pt{c}_{b}")
                        nc.tensor.matmul(pt[:, :],
                                         w_sb[:, c * R:(c + 1) * R],
                                         xs[:, jj * S + b * TOK:
                                            jj * S + (b + 1) * TOK])
                        dst = ys[:, j * S + b * TOK:j * S + (b + 1) * TOK]
                        if ncopy % 2 == 0:
                            nc.vector.tensor_copy(dst, pt[:, :])
                        else:
                            nc.scalar.copy(out=dst, in_=pt[:, :])
                        ncopy += 1
                nc.scalar.dma_start(
                    out=yt.ap()[:, c0 * S:(c0 + ng) * S], in_=ys[:, :])
    nc.compile()
    return nc


def kernel(x, sign1, sign2, perm):
    global _last_run
    x = np.asarray(x)
    sign1 = np.asarray(sign1)
    sign2 = np.asarray(sign2)
    perm = np.asarray(perm)

    if not _nc_cache:
        _nc_cache.append(_build_nc())
    nc = _nc_cache[0]

    key = (perm.tobytes(), sign1.tobytes(), sign2.tobytes())
    if key not in _w_cache:
        _w_cache[key] = _build_weights(perm, sign1, sign2)
    w_p = _w_cache[key]

    # host staging: per-(token, 64-block) absmax scale, quantize to e3m4,
    # transpose to [dim, tok] and pack for contiguous superstep DMAs
    in_maps = []
    scales = []
    for b in range(B):
        xr = x[b].astype(np.float32).reshape(S, D // BLOCK, BLOCK)
        amax = np.abs(xr).max(axis=2, keepdims=True)
        sc = np.maximum(amax / QMAX, 1e-8).astype(np.float32)
        xq = (xr / sc).astype(ml_dtypes.float8_e3m4).reshape(S, D)
        scales.append(sc.reshape(S, D // BLOCK))
        xt_dev = np.ascontiguousarray(
            xq.reshape(S, C // SIN, SIN, R).transpose(1, 3, 2, 0)
        ).reshape(C // SIN, R, SIN * S)
        in_maps.append({"xt": xt_dev, "w": w_p})

    from concourse.bass_utils import run_bass_kernel_spmd
    res = run_bass_kernel_spmd(nc, in_maps, list(range(NCORES)))
    _last_run = (nc, in_maps)

    perm64 = perm.astype(np.int64)
    blk = perm64 >> 6                      # source 64-block of output col j
    out = np.empty((B, S, D), dtype=np.float32)
    for b in range(B):
        yt_dev = np.asarray(res.results[b]["yt"]).reshape(R, C, S)
        zT = yt_dev.transpose(1, 0, 2).reshape(D, S)
        g = zT[perm64].astype(np.float32)          # [Dout, S]
        scT = np.ascontiguousarray(scales[b].T)    # [D//BLOCK, S]
        g *= scT[blk]
        out[b] = g.T
    return out
